# revision 15
# baseline (speedup 1.0000x reference)
"""Trainium2 Bass kernel for nn_Attention_81655918231876.

RoPE attention with positional bias, 8 heads / dim_head 64, b=2, n=2048, dim=512.
Sharding: head-parallel across 8 cores. Core h computes head h for BOTH batches
and ships the per-head attention output O_h^T (bf16) plus softmax row sums
(f32); the host applies 1/rowsum, the w_out projection, and the head sum.

Design notes (all-bf16 matmul path; margin under the 2e-2 gate):
  - Steady state is exp-paced: Scalar does 64 x exp([128,1024]) ~ 70-80us and
    every other engine hides under it. The schedule's job is (a) first exp
    fires as early as possible, (b) Scalar never starves, (c) short tail.
  - DMA priority order: wall, x cols the dense prelude needs (b0/b1 key+q
    chunks), cos/sin table, eb j-blocks 0-3, rest of x, eb 4-15. The 8MB eb
    table must not starve the 2MB of x the first projections need.
  - Projections: stationary weight blocks [q|qrot], [k|krot], [v|pad] in bf16,
    moving x^T chunks, two 512-token chunks per PSUM group; RoPE combine is
    ONE fused DVE mul [128,1024] against the stacked cos/sin table (bf16 out)
    plus one bf16 add at 2x rate.
  - S = q k^T as plain bf16 K=64 matmuls; the two batches' matmuls use PE
    row groups 0/64 (tile_position via base partition) and run CONCURRENTLY.
  - exp(S) on ScalarE -> bf16; bias multiply exp(S)*exp(bias) on DVE at 2x
    bf16 rate (one [128,2,512] op; the eb block is a stride-0 broadcast).
  - P V accumulated with an extra ones-column in V so row 64 of O^T is the
    softmax row sum (M=65); V natural layout built by one DMA xbar transpose
    per batch (destination blocks 32B-aligned, VSTRIDE=80).
  - Software pipeline: minimal dense prelude (k + quarter-0 q projections
    only), then quarter-0's S/exp/mult overlaps the projection tail via a
    filler queue; each quarter's PV + copies + DMAs drain as fillers inside
    the NEXT quarter's exp-paced loop, so the PE stream stays dense.
"""

import numpy as np
import ml_dtypes
import sys

sys.path.insert(0, "/opt/trn_rl_repo")

HEADS = 8
DIM_HEAD = 64
ROPE_THETA = 10000.0
B, N, DIM = 2, 2048, 512
# per-j-block column stride in vsb: 64 V cols + 1 ones col + pad. Must keep
# every block's byte offset 32B-aligned: the DMA xbar transpose writes in
# 16-element (bf16) groups and silently corrupts unaligned destinations.
VSTRIDE = 80

_compiled = None
_DEBUG = False


def _build():
    import concourse.bass as bass
    import concourse.tile as tile
    from concourse import bacc, mybir

    f32 = mybir.dt.float32
    bf16 = mybir.dt.bfloat16
    Exp = mybir.ActivationFunctionType.Exp

    nc = bacc.Bacc(None, target_bir_lowering=False, debug=False)
    xt = nc.dram_tensor("xt", [DIM, 2 * N], bf16, kind="ExternalInput")
    wall = nc.dram_tensor("wall", [DIM, 384], bf16, kind="ExternalInput")
    cs2 = nc.dram_tensor("cs2", [128, N], f32, kind="ExternalInput")
    ebt = nc.dram_tensor("ebt", [N, N], bf16, kind="ExternalInput")
    oto = nc.dram_tensor("oto", [4 * B, 64, 512], bf16, kind="ExternalOutput")
    rsum = nc.dram_tensor("rsum", [4 * B, 512], f32, kind="ExternalOutput")
    if _DEBUG:
        dbg_qkv = nc.dram_tensor("dbg_qkv", [3, 128, N], bf16, kind="ExternalOutput")
        dbg_vsb = nc.dram_tensor("dbg_vsb", [B, 128, 16 * VSTRIDE], bf16, kind="ExternalOutput")

    with tile.TileContext(nc) as tc:
        with (
            tc.tile_pool(name="singles", bufs=1) as singles,
            tc.tile_pool(name="t12p", bufs=2) as t12p,
            tc.tile_pool(name="ptsp", bufs=10) as ptsp,
            tc.tile_pool(name="ptp", bufs=20) as ptp,
            tc.tile_pool(name="rrp", bufs=2) as rrp,
            tc.tile_pool(name="otp", bufs=2) as otp,
            tc.tile_pool(name="psS", bufs=2, space="PSUM") as psS,
        ):
            # ---- inputs, in bandwidth-priority order ----
            # DMA triggers issue serially (~650ns each on the Sync seq), so
            # few+ordered triggers matter. The dense prelude needs: wall,
            # x token-chunks 0 of both batches (cols 0-1023 / 2048-3071),
            # cos/sin. eb j-blocks 0-3 must land by the time quarter-0's
            # mult loop starts; the rest of eb follows the remaining x.
            wall_sb = singles.tile([128, 4 * 384], bf16, tag="wall", name="wall_sb")
            nc.sync.dma_start(
                out=wall_sb.rearrange("p (k c) -> p k c", k=4),
                in_=wall[:, :].rearrange("(k p) c -> p k c", k=4))
            wl = [wall_sb[:, 384 * k:384 * (k + 1)] for k in range(4)]
            xb = [singles.tile([128, 2 * N], bf16, tag=f"xb{k}", name=f"xb{k}") for k in range(4)]

            def xload(k, lo, hi):
                nc.sync.dma_start(
                    out=xb[k][:, lo:hi],
                    in_=xt[128 * k:128 * (k + 1), lo:hi])

            cs_sb = singles.tile([128, N], f32, tag="cs", name="cs_sb")
            eb_sb = singles.tile([128, 16 * N], bf16, tag="eb", name="eb_sb")

            def ebload(j0, nj):
                nc.sync.dma_start(
                    out=eb_sb[:, N * j0:N * (j0 + nj)].rearrange(
                        "p (j c) -> p j c", j=nj),
                    in_=ebt[128 * j0:128 * (j0 + nj), :].rearrange(
                        "(j p) c -> p j c", j=nj))

            # priority: exactly the token-chunk-0 columns (both batches)
            # the two dense prelude groups need, then the quarter-0 part of
            # the cos/sin table.
            for k in range(4):
                xload(k, 0, 512)
                xload(k, 2048, 2560)
            nc.sync.dma_start(out=cs_sb[:, 0:512], in_=cs2[:, 0:512])
            nc.sync.dma_start(out=cs_sb[:, 512:N], in_=cs2[:, 512:N])
            for k in range(4):
                xload(k, 512, 1024)
                xload(k, 2560, 3072)
            ebload(0, 4)
            for k in range(4):
                xload(k, 1024, 2048)
                xload(k, 3072, 4096)
            ebload(4, 4)
            ebload(8, 4)
            ebload(12, 4)

            qb = singles.tile([128, N], bf16, tag="qb", name="qb")
            kb = singles.tile([128, N], bf16, tag="kb", name="kb")
            vt = singles.tile([128, N], bf16, tag="vt", name="vt")
            vsb = [singles.tile([128, 16 * VSTRIDE], bf16, tag=f"vsb{b}", name=f"vsb{b}")
                   for b in range(B)]
            for b in range(B):
                nc.vector.memset(vsb[b], 1.0)

            # ---- pipelined projection + attention ----
            from collections import deque
            fillers = deque()

            def emit_fill(n):
                for _ in range(n):
                    if fillers:
                        fillers.popleft()()

            def rope_ops(mt, c, ptile):
                """RoPE combine for group (mt, c): ptile holds both batches'
                tokens 512c..512c+511 ([128 rows plain|rot, (b, 512)]). Two
                muls against the cos/sin table (bf16 out, stride-0 batch
                broadcast), then one bf16 2x-rate add per batch. t1/t2 sit
                at base partition 0 (TensorTensor needs equal SBUF input
                base partitions)."""
                tok = 512 * c
                dst = qb if mt == 0 else kb
                t1 = t12p.tile([64, 1024], bf16, tag="t1",
                               name=f"t1_{mt}_{c}")
                t2 = t12p.tile([64, 1024], bf16, tag="t2",
                               name=f"t2_{mt}_{c}")
                csb = cs_sb[:, tok:tok + 512].unsqueeze(1)
                nc.vector.tensor_mul(
                    t1.rearrange("p (r c) -> p r c", r=2),
                    ptile[0:64, :].rearrange("p (r c) -> p r c", r=2),
                    csb[0:64].broadcast_to((64, 2, 512)))
                nc.vector.tensor_mul(
                    t2.rearrange("p (r c) -> p r c", r=2),
                    ptile[64:128, :].rearrange("p (r c) -> p r c", r=2),
                    csb[64:128].broadcast_to((64, 2, 512)))
                for b in range(B):
                    nc.vector.tensor_add(
                        dst[64 * b:64 * b + 64, tok:tok + 512],
                        t1[:, 512 * b:512 * (b + 1)],
                        t2[:, 512 * b:512 * (b + 1)])

            def vcopy_op(c, ptile):
                tok = 512 * c
                for b in range(B):
                    nc.vector.tensor_copy(
                        vt[64 * b:64 * b + 64, tok:tok + 512],
                        ptile[0:64, 512 * b:512 * (b + 1)])

            def proj_matmuls(tile, mt, c, k):
                for ci, cc in enumerate((c, c + 4)):
                    nc.tensor.matmul(
                        tile[:, 512 * ci:512 * (ci + 1)],
                        wl[k][:, 128 * mt:128 * (mt + 1)],
                        xb[k][:, 512 * cc:512 * (cc + 1)],
                        start=(k == 0), stop=(k == 3),
                    )

            def proj_group_closures(mt, c, pool=None):
                out = []

                def mms(k):
                    def f():
                        tile = proj_group_closures.tiles.get((mt, c))
                        if tile is None:
                            p = psP if pool is None else pool
                            tile = p.tile([128, 1024], f32, tag="pp",
                                          name=f"pp_{mt}_{c}")
                            proj_group_closures.tiles[(mt, c)] = tile
                        proj_matmuls(tile, mt, c, k)
                    return f
                for k in range(4):
                    out.append(mms(k))

                if mt < 2:
                    def rope():
                        tile = proj_group_closures.tiles.pop((mt, c))
                        rope_ops(mt, c, tile)
                    out.append(rope)
                else:
                    def vcopy():
                        tile = proj_group_closures.tiles.pop((mt, c))
                        vcopy_op(c, tile)
                    out.append(vcopy)
                return out
            proj_group_closures.tiles = {}

            def proj_group(mt, c):
                """Dense variant: emit the 8 matmuls now, return the rope/v
                closures to drain while the next group's matmuls stream."""
                tile = psP.tile([128, 1024], f32, tag="pp",
                                name=f"pp_{mt}_{c}")
                for k in range(4):
                    proj_matmuls(tile, mt, c, k)
                if mt < 2:
                    return [lambda: rope_ops(mt, c, tile)]
                return [lambda: vcopy_op(c, tile)]

            def vtrans(b):
                def f():
                    dst = vsb[b].rearrange("p (j c) -> p j c",
                                           c=VSTRIDE)[:, :, 0:64]
                    nc.sync.dma_start_transpose(dst, vt[64 * b:64 * b + 64, :])
                return f

            pt_store = {}

            def s_loop(q, fill_per_step):
                """16-step S/exp/mult loop for quarter q; P tiles are kept
                for the next quarter's PV fillers. The S matmul for step
                j+1 is emitted BEFORE step j's fillers, so exp(j+1) never
                queues behind filler matmuls on the in-order PE."""
                i0 = 512 * q
                budgets = (fill_per_step if isinstance(fill_per_step, list)
                           else [fill_per_step] * 16)

                def s_mm(j):
                    s_ps = psS.tile([128, 1024], f32, tag="s",
                                    name=f"s_{q}_{j}")
                    for b in range(B):
                        nc.tensor.matmul(
                            s_ps[:, 512 * b:512 * (b + 1)],
                            kb[64 * b:64 * b + 64, 128 * j:128 * (j + 1)],
                            qb[64 * b:64 * b + 64, i0:i0 + 512],
                            start=True, stop=True,
                        )
                    return s_ps

                s_ps = s_mm(0)
                for j in range(16):
                    pts = ptsp.tile([128, 1024], bf16, tag="pts",
                                    name=f"pts_{q}_{j}")
                    nc.scalar.activation(pts, s_ps, Exp)
                    if j + 1 < 16:
                        s_ps = s_mm(j + 1)
                    pt = ptp.tile([128, 1024], bf16, tag="pt",
                                  name=f"pt_{q}_{j}")
                    ebs = eb_sb[:, N * j + i0:N * j + i0 + 512]
                    nc.vector.tensor_mul(
                        pt.rearrange("p (r c) -> p r c", r=2),
                        pts.rearrange("p (r c) -> p r c", r=2),
                        ebs.unsqueeze(1).broadcast_to((128, 2, 512)))
                    pt_store[(q, j)] = pt
                    emit_fill(budgets[j])

            def quarter_drain_closures(q):
                """PV + rowsum/O copies + output DMA for quarter q,
                as closures to interleave into the next quarter's loop."""
                ots = [psO.tile([65, 512], f32, tag=f"o{b}",
                                name=f"ot_{b}_{q}") for b in range(B)]
                out_cl = []

                def mk_pv(j):
                    def f():
                        for b in range(B):
                            nc.tensor.matmul(
                                ots[b],
                                vsb[b][:, VSTRIDE * j:VSTRIDE * j + 65],
                                pt_store.pop((q, j))[:, 512 * b:512 * (b + 1)]
                                if b == B - 1 else
                                pt_store[(q, j)][:, 512 * b:512 * (b + 1)],
                                start=(j == 0), stop=(j == 15),
                            )
                    return f
                out_cl += [mk_pv(j) for j in range(16)]

                def mk_copies(b):
                    ot = ots[b]
                    rs = rrp.tile([1, 512], f32, tag="rs", name=f"rs_{b}_{q}")
                    otsb = otp.tile([64, 512], bf16, tag=f"otsb{b}",
                                    name=f"otsb_{b}_{q}")

                    def f():
                        nc.vector.tensor_copy(rs, ot[64:65, :])
                        nc.sync.dma_start(
                            out=rsum[4 * b + q:4 * b + q + 1, :], in_=rs)
                        nc.vector.tensor_copy(otsb, ot[0:64, :])
                        nc.sync.dma_start(out=oto[4 * b + q, :, :], in_=otsb)
                    return f
                out_cl += [mk_copies(b) for b in range(B)]
                return out_cl

            # ---- phase 1 ----
            # Dense prelude covers only what quarter 0 needs up front:
            # group (1,0) = keys 0-511 both batches (S j-blocks 0-3) and
            # group (0,0) = quarter-0 q both batches. Everything else flows
            # in as fillers inside the quarter loops.
            with tc.tile_pool(name="psP", bufs=2, space="PSUM") as psP:
                pending = []
                for mt, c in ((1, 0), (0, 0)):
                    for p in pending:
                        p()
                    pending = proj_group(mt, c)
                for p in pending:
                    p()
                # filler order fixes rope deadlines: k(c) before S j=4c,
                # q(1) before s_loop(1); v groups + transposes before the
                # PV fillers that run inside s_loop(1).
                for mt, c in ((1, 1), (0, 1), (1, 2), (1, 3),
                              (2, 0), (2, 1), (2, 2), (2, 3)):
                    fillers.extend(proj_group_closures(mt, c))
                fillers.append(vtrans(0))
                fillers.append(vtrans(1))
                # pre-pump a few filler matmuls: S(0,0) head-of-line blocks
                # the in-order PE queue on the DVE rope chain; these k
                # matmuls only need resident x/weights and fill that window
                emit_fill(4)
                s_loop(0, 2)
                emit_fill(len(fillers))

            if _DEBUG:
                nc.sync.dma_start(out=dbg_qkv[0, :, :], in_=qb)
                nc.sync.dma_start(out=dbg_qkv[1, :, :], in_=kb)
                nc.sync.dma_start(out=dbg_qkv[2, :, :], in_=vt)
                for b in range(B):
                    nc.sync.dma_start(out=dbg_vsb[b, :, :], in_=vsb[b])

            # ---- phase 2 ----
            with (
                tc.tile_pool(name="psO", bufs=1, space="PSUM") as psO,
                tc.tile_pool(name="psQ", bufs=1, space="PSUM") as psQ,
            ):
                fillers.extend(proj_group_closures(0, 2, pool=psQ))
                fillers.extend(quarter_drain_closures(0))
                s_loop(1, 3)
                fillers.extend(proj_group_closures(0, 3, pool=psQ))
                fillers.extend(quarter_drain_closures(1))
                s_loop(2, 2)
                fillers.extend(quarter_drain_closures(2))
                fillers.extend(quarter_drain_closures(3))
                s_loop(3, 2)
                emit_fill(len(fillers))

    nc.compile()
    return nc


def _host_inputs(x, pos_bias, w_qkv, w_out):
    """Build the per-core input maps (head-parallel sharding)."""
    bf = ml_dtypes.bfloat16
    x = np.asarray(x, dtype=np.float32)
    pos_bias = np.asarray(pos_bias, dtype=np.float32)
    w_qkv = np.asarray(w_qkv, dtype=np.float32)
    w_out = np.asarray(w_out, dtype=np.float32)
    hidden = HEADS * DIM_HEAD

    xt = np.ascontiguousarray(
        np.concatenate([x[0].T, x[1].T], axis=1)).astype(bf)  # [512, 4096]

    inv_freq = 1.0 / (ROPE_THETA ** (np.arange(0, DIM_HEAD, 2, dtype=np.float64) / DIM_HEAD))
    freqs = np.arange(N, dtype=np.float64)[:, None] * inv_freq[None, :]
    freqs = np.repeat(freqs, 2, axis=-1)  # [n, 64]
    cosT = np.cos(freqs).T.astype(np.float32)
    sinT = np.sin(freqs).T.astype(np.float32)
    cs2 = np.ascontiguousarray(np.concatenate([cosT, sinT], axis=0))  # [128, n]

    def rot_cols(w):
        wr = np.empty_like(w)
        wr[:, 0::2] = -w[:, 1::2]
        wr[:, 1::2] = w[:, 0::2]
        return wr

    scale = DIM_HEAD ** -0.5
    in_maps = []
    for h in range(HEADS):
        wq = w_qkv[:, h * 64:(h + 1) * 64] * scale
        wk = w_qkv[:, hidden + h * 64:hidden + (h + 1) * 64]
        wvh = w_qkv[:, 2 * hidden + h * 64:2 * hidden + (h + 1) * 64]
        wall = np.ascontiguousarray(
            np.concatenate(
                [wq, rot_cols(wq), wk, rot_cols(wk), wvh,
                 np.zeros((DIM, 64), dtype=np.float32)], axis=1)
        ).astype(bf)  # [512, 384]
        in_maps.append({
            "xt": xt,
            "wall": wall,
            "cs2": cs2,
            "ebt": np.ascontiguousarray(np.exp(pos_bias[h]).T).astype(bf),
        })
    return in_maps


def kernel(x, pos_bias, w_qkv, w_out, _want_trace=False):
    global _compiled
    from concourse.bass_utils import run_bass_kernel_spmd

    if _compiled is None:
        _compiled = _build()
    in_maps = _host_inputs(x, pos_bias, w_qkv, w_out)
    res = run_bass_kernel_spmd(
        _compiled, in_maps, core_ids=list(range(HEADS)), trace=_want_trace
    )
    w_out = np.asarray(w_out, dtype=np.float32)
    y = np.zeros((B, N, DIM), dtype=np.float32)
    for h, r in enumerate(res.results):
        rs = np.asarray(r["rsum"]).reshape(B, N)
        # oto: [4b+q, 64 d, 512 tok] -> O [B, N, 64]
        ot = np.asarray(r["oto"]).astype(np.float32)
        O = ot.reshape(B, 4, 64, 512).transpose(0, 1, 3, 2).reshape(B, N, 64)
        y += (O / rs[:, :, None]) @ w_out[h * 64:(h + 1) * 64, :]
    if _want_trace:
        kernel._last_results = res
    return y


# revision 19
# speedup vs baseline: 1.0109x; 1.0109x over previous
"""Trainium2 Bass kernel for nn_Attention_81655918231876.

RoPE attention with positional bias, 8 heads / dim_head 64, b=2, n=2048, dim=512.
Sharding: head-parallel across 8 cores. Core h computes head h for BOTH batches
and ships the per-head attention output O_h^T (bf16) plus softmax row sums
(f32); the host applies 1/rowsum, the w_out projection, and the head sum.

Design notes (all-bf16 matmul path; margin under the 2e-2 gate):
  - Steady state is exp-paced: Scalar does 64 x exp([128,1024]) ~ 70-80us and
    every other engine hides under it. The schedule's job is (a) first exp
    fires as early as possible, (b) Scalar never starves, (c) short tail.
  - DMA priority order: wall, x cols the dense prelude needs (b0/b1 key+q
    chunks), cos/sin table, eb j-blocks 0-3, rest of x, eb 4-15. The 8MB eb
    table must not starve the 2MB of x the first projections need.
  - Projections: stationary weight blocks [q|qrot], [k|krot], [v|pad] in bf16,
    moving x^T chunks, two 512-token chunks per PSUM group; RoPE combine is
    ONE fused DVE mul [128,1024] against the stacked cos/sin table (bf16 out)
    plus one bf16 add at 2x rate.
  - S = q k^T as plain bf16 K=64 matmuls; the two batches' matmuls use PE
    row groups 0/64 (tile_position via base partition) and run CONCURRENTLY.
  - exp(S) on ScalarE -> bf16; bias multiply exp(S)*exp(bias) on DVE at 2x
    bf16 rate (one [128,2,512] op; the eb block is a stride-0 broadcast).
  - P V accumulated with an extra ones-column in V so row 64 of O^T is the
    softmax row sum (M=65); V natural layout built by one DMA xbar transpose
    per batch (destination blocks 32B-aligned, VSTRIDE=80).
  - Software pipeline: minimal dense prelude (k + quarter-0 q projections
    only), then quarter-0's S/exp/mult overlaps the projection tail via a
    filler queue; each quarter's PV + copies + DMAs drain as fillers inside
    the NEXT quarter's exp-paced loop, so the PE stream stays dense.
"""

import numpy as np
import ml_dtypes
import sys

sys.path.insert(0, "/opt/trn_rl_repo")

HEADS = 8
DIM_HEAD = 64
ROPE_THETA = 10000.0
B, N, DIM = 2, 2048, 512
# per-j-block column stride in vsb: 64 V cols + 1 ones col + pad. Must keep
# every block's byte offset 32B-aligned: the DMA xbar transpose writes in
# 16-element (bf16) groups and silently corrupts unaligned destinations.
VSTRIDE = 80

_compiled = None
_DEBUG = False


def _build():
    import concourse.bass as bass
    import concourse.tile as tile
    from concourse import bacc, mybir

    f32 = mybir.dt.float32
    bf16 = mybir.dt.bfloat16
    Exp = mybir.ActivationFunctionType.Exp

    nc = bacc.Bacc(None, target_bir_lowering=False, debug=False)
    xt = nc.dram_tensor("xt", [DIM, 2 * N], bf16, kind="ExternalInput")
    wall = nc.dram_tensor("wall", [DIM, 384], bf16, kind="ExternalInput")
    cs2 = nc.dram_tensor("cs2", [128, N], f32, kind="ExternalInput")
    ebt = nc.dram_tensor("ebt", [N, N], bf16, kind="ExternalInput")
    oto = nc.dram_tensor("oto", [4 * B, 64, 512], bf16, kind="ExternalOutput")
    rsum = nc.dram_tensor("rsum", [4 * B, 512], f32, kind="ExternalOutput")
    if _DEBUG:
        dbg_qkv = nc.dram_tensor("dbg_qkv", [3, 128, N], bf16, kind="ExternalOutput")
        dbg_vsb = nc.dram_tensor("dbg_vsb", [B, 128, 16 * VSTRIDE], bf16, kind="ExternalOutput")

    with tile.TileContext(nc) as tc:
        with (
            tc.tile_pool(name="singles", bufs=1) as singles,
            tc.tile_pool(name="t12p", bufs=2) as t12p,
            tc.tile_pool(name="ptsp", bufs=10) as ptsp,
            tc.tile_pool(name="ptp", bufs=20) as ptp,
            tc.tile_pool(name="rrp", bufs=2) as rrp,
            tc.tile_pool(name="otp", bufs=2) as otp,
            tc.tile_pool(name="psS", bufs=2, space="PSUM") as psS,
        ):
            # ---- inputs, in bandwidth-priority order ----
            # DMA triggers issue serially (~650ns each on the Sync seq), so
            # few+ordered triggers matter. The dense prelude needs: wall,
            # x token-chunks 0 of both batches (cols 0-1023 / 2048-3071),
            # cos/sin. eb j-blocks 0-3 must land by the time quarter-0's
            # mult loop starts; the rest of eb follows the remaining x.
            wall_sb = singles.tile([128, 4 * 384], bf16, tag="wall", name="wall_sb")
            nc.sync.dma_start(
                out=wall_sb.rearrange("p (k c) -> p k c", k=4),
                in_=wall[:, :].rearrange("(k p) c -> p k c", k=4))
            wl = [wall_sb[:, 384 * k:384 * (k + 1)] for k in range(4)]
            xb = [singles.tile([128, 2 * N], bf16, tag=f"xb{k}", name=f"xb{k}") for k in range(4)]

            def xload(k, lo, hi):
                nc.sync.dma_start(
                    out=xb[k][:, lo:hi],
                    in_=xt[128 * k:128 * (k + 1), lo:hi])

            cs_sb = singles.tile([128, N], f32, tag="cs", name="cs_sb")
            eb_sb = singles.tile([128, 16 * N], bf16, tag="eb", name="eb_sb")

            def ebload(j0, nj):
                nc.sync.dma_start(
                    out=eb_sb[:, N * j0:N * (j0 + nj)].rearrange(
                        "p (j c) -> p j c", j=nj),
                    in_=ebt[128 * j0:128 * (j0 + nj), :].rearrange(
                        "(j p) c -> p j c", j=nj))

            # priority: exactly the token-chunk-0 columns (both batches)
            # the two dense prelude groups need, then the quarter-0 part of
            # the cos/sin table.
            for k in range(4):
                xload(k, 0, 512)
                xload(k, 2048, 2560)
            nc.sync.dma_start(out=cs_sb[:, 0:512], in_=cs2[:, 0:512])
            nc.sync.dma_start(out=cs_sb[:, 512:N], in_=cs2[:, 512:N])
            for k in range(4):
                xload(k, 512, 1024)
                xload(k, 2560, 3072)
            ebload(0, 4)
            for k in range(4):
                xload(k, 1024, 2048)
                xload(k, 3072, 4096)
            ebload(4, 4)
            ebload(8, 4)
            ebload(12, 4)

            qb = singles.tile([128, N], bf16, tag="qb", name="qb")
            kb = singles.tile([128, N], bf16, tag="kb", name="kb")
            vt = singles.tile([128, N], bf16, tag="vt", name="vt")
            vsb = [singles.tile([128, 16 * VSTRIDE], bf16, tag=f"vsb{b}", name=f"vsb{b}")
                   for b in range(B)]
            for b in range(B):
                nc.vector.memset(vsb[b], 1.0)

            # ---- pipelined projection + attention ----
            from collections import deque
            fillers = deque()

            def emit_fill(n):
                for _ in range(n):
                    if fillers:
                        fillers.popleft()()

            def rope_ops(mt, c, ptile):
                """RoPE combine for group (mt, c): ptile holds both batches'
                tokens 512c..512c+511 ([128 rows plain|rot, (b, 512)]). Two
                muls against the cos/sin table (bf16 out, stride-0 batch
                broadcast), then one bf16 2x-rate add per batch. t1/t2 sit
                at base partition 0 (TensorTensor needs equal SBUF input
                base partitions)."""
                tok = 512 * c
                dst = qb if mt == 0 else kb
                t1 = t12p.tile([64, 1024], bf16, tag="t1",
                               name=f"t1_{mt}_{c}")
                t2 = t12p.tile([64, 1024], bf16, tag="t2",
                               name=f"t2_{mt}_{c}")
                csb = cs_sb[:, tok:tok + 512].unsqueeze(1)
                nc.vector.tensor_mul(
                    t1.rearrange("p (r c) -> p r c", r=2),
                    ptile[0:64, :].rearrange("p (r c) -> p r c", r=2),
                    csb[0:64].broadcast_to((64, 2, 512)))
                nc.vector.tensor_mul(
                    t2.rearrange("p (r c) -> p r c", r=2),
                    ptile[64:128, :].rearrange("p (r c) -> p r c", r=2),
                    csb[64:128].broadcast_to((64, 2, 512)))
                for b in range(B):
                    nc.vector.tensor_add(
                        dst[64 * b:64 * b + 64, tok:tok + 512],
                        t1[:, 512 * b:512 * (b + 1)],
                        t2[:, 512 * b:512 * (b + 1)])

            def vcopy_op(c, ptile):
                tok = 512 * c
                for b in range(B):
                    nc.vector.tensor_copy(
                        vt[64 * b:64 * b + 64, tok:tok + 512],
                        ptile[0:64, 512 * b:512 * (b + 1)])

            def proj_matmuls(tile, mt, c, k):
                for ci, cc in enumerate((c, c + 4)):
                    nc.tensor.matmul(
                        tile[:, 512 * ci:512 * (ci + 1)],
                        wl[k][:, 128 * mt:128 * (mt + 1)],
                        xb[k][:, 512 * cc:512 * (cc + 1)],
                        start=(k == 0), stop=(k == 3),
                    )

            def proj_group_closures(mt, c, pool=None):
                out = []

                def mms(k):
                    def f():
                        tile = proj_group_closures.tiles.get((mt, c))
                        if tile is None:
                            p = psP if pool is None else pool
                            tile = p.tile([128, 1024], f32, tag="pp",
                                          name=f"pp_{mt}_{c}")
                            proj_group_closures.tiles[(mt, c)] = tile
                        proj_matmuls(tile, mt, c, k)
                    return f
                for k in range(4):
                    out.append(mms(k))

                if mt < 2:
                    def rope():
                        tile = proj_group_closures.tiles.pop((mt, c))
                        rope_ops(mt, c, tile)
                    out.append(rope)
                else:
                    def vcopy():
                        tile = proj_group_closures.tiles.pop((mt, c))
                        vcopy_op(c, tile)
                    out.append(vcopy)
                return out
            proj_group_closures.tiles = {}

            def proj_group(mt, c):
                """Dense variant: emit the 8 matmuls now, return the rope/v
                closures to drain while the next group's matmuls stream."""
                tile = psP.tile([128, 1024], f32, tag="pp",
                                name=f"pp_{mt}_{c}")
                for k in range(4):
                    proj_matmuls(tile, mt, c, k)
                if mt < 2:
                    return [lambda: rope_ops(mt, c, tile)]
                return [lambda: vcopy_op(c, tile)]

            def vtrans(b):
                def f():
                    dst = vsb[b].rearrange("p (j c) -> p j c",
                                           c=VSTRIDE)[:, :, 0:64]
                    nc.sync.dma_start_transpose(dst, vt[64 * b:64 * b + 64, :])
                return f

            pt_store = {}
            next_s = {}

            def s_mm(q, j):
                s_ps = psS.tile([128, 1024], f32, tag="s",
                                name=f"s_{q}_{j}")
                for b in range(B):
                    nc.tensor.matmul(
                        s_ps[:, 512 * b:512 * (b + 1)],
                        kb[64 * b:64 * b + 64, 128 * j:128 * (j + 1)],
                        qb[64 * b:64 * b + 64, 512 * q:512 * q + 512],
                        start=True, stop=True,
                    )
                return s_ps

            def s_loop(q, fill_per_step):
                """16-step S/exp/mult loop for quarter q; P tiles are kept
                for the next quarter's PV fillers. The S matmul for step
                j+1 is emitted BEFORE step j's fillers, so exp(j+1) never
                queues behind filler matmuls on the in-order PE; the NEXT
                quarter's first S matmul is emitted right after the last
                exp so the cross-loop boundary never starves Scalar."""
                budgets = (fill_per_step if isinstance(fill_per_step, list)
                           else [fill_per_step] * 16)
                s_ps = next_s.pop(q, None)
                if s_ps is None:
                    s_ps = s_mm(q, 0)
                for j in range(16):
                    pts = ptsp.tile([128, 1024], bf16, tag="pts",
                                    name=f"pts_{q}_{j}")
                    nc.scalar.activation(pts, s_ps, Exp)
                    if j + 1 < 16:
                        s_ps = s_mm(q, j + 1)
                    elif q + 1 <= 3:
                        next_s[q + 1] = s_mm(q + 1, 0)
                    pt = ptp.tile([128, 1024], bf16, tag="pt",
                                  name=f"pt_{q}_{j}")
                    ebs = eb_sb[:, N * j + 512 * q:N * j + 512 * q + 512]
                    nc.vector.tensor_mul(
                        pt.rearrange("p (r c) -> p r c", r=2),
                        pts.rearrange("p (r c) -> p r c", r=2),
                        ebs.unsqueeze(1).broadcast_to((128, 2, 512)))
                    pt_store[(q, j)] = pt
                    emit_fill(budgets[j])

            def quarter_drain_closures(q):
                """PV + rowsum/O copies + output DMA for quarter q,
                as closures to interleave into the next quarter's loop."""
                ots = [psO.tile([65, 512], f32, tag=f"o{b}",
                                name=f"ot_{b}_{q}") for b in range(B)]
                out_cl = []

                def mk_pv(j):
                    def f():
                        for b in range(B):
                            nc.tensor.matmul(
                                ots[b],
                                vsb[b][:, VSTRIDE * j:VSTRIDE * j + 65],
                                pt_store.pop((q, j))[:, 512 * b:512 * (b + 1)]
                                if b == B - 1 else
                                pt_store[(q, j)][:, 512 * b:512 * (b + 1)],
                                start=(j == 0), stop=(j == 15),
                            )
                    return f
                out_cl += [mk_pv(j) for j in range(16)]

                def mk_copies(b):
                    ot = ots[b]
                    rs = rrp.tile([1, 512], f32, tag="rs", name=f"rs_{b}_{q}")
                    otsb = otp.tile([64, 512], bf16, tag=f"otsb{b}",
                                    name=f"otsb_{b}_{q}")

                    def f():
                        nc.vector.tensor_copy(rs, ot[64:65, :])
                        nc.sync.dma_start(
                            out=rsum[4 * b + q:4 * b + q + 1, :], in_=rs)
                        nc.vector.tensor_copy(otsb, ot[0:64, :])
                        nc.sync.dma_start(out=oto[4 * b + q, :, :], in_=otsb)
                    return f
                out_cl += [mk_copies(b) for b in range(B)]
                return out_cl

            # ---- phase 1 ----
            # Dense prelude covers only what quarter 0 needs up front:
            # group (1,0) = keys 0-511 both batches (S j-blocks 0-3) and
            # group (0,0) = quarter-0 q both batches. Everything else flows
            # in as fillers inside the quarter loops.
            with tc.tile_pool(name="psP", bufs=2, space="PSUM") as psP:
                pending = []
                for mt, c in ((1, 0), (0, 0)):
                    for p in pending:
                        p()
                    pending = proj_group(mt, c)
                for p in pending:
                    p()
                # filler order fixes rope deadlines: k(c) before S j=4c,
                # q(1) before s_loop(1); v groups + transposes before the
                # PV fillers that run inside s_loop(1).
                for mt, c in ((1, 1), (0, 1), (1, 2), (1, 3),
                              (2, 0), (2, 1), (2, 2), (2, 3)):
                    fillers.extend(proj_group_closures(mt, c))
                fillers.append(vtrans(0))
                fillers.append(vtrans(1))
                # pre-pump a few filler matmuls: S(0,0) head-of-line blocks
                # the in-order PE queue on the DVE rope chain; these k
                # matmuls only need resident x/weights and fill that window
                emit_fill(5)
                s_loop(0, 3)
                emit_fill(len(fillers))

            if _DEBUG:
                nc.sync.dma_start(out=dbg_qkv[0, :, :], in_=qb)
                nc.sync.dma_start(out=dbg_qkv[1, :, :], in_=kb)
                nc.sync.dma_start(out=dbg_qkv[2, :, :], in_=vt)
                for b in range(B):
                    nc.sync.dma_start(out=dbg_vsb[b, :, :], in_=vsb[b])

            # ---- phase 2 ----
            with (
                tc.tile_pool(name="psO", bufs=1, space="PSUM") as psO,
                tc.tile_pool(name="psQ", bufs=1, space="PSUM") as psQ,
            ):
                fillers.extend(proj_group_closures(0, 2, pool=psQ))
                fillers.extend(quarter_drain_closures(0))
                s_loop(1, 2)
                fillers.extend(proj_group_closures(0, 3, pool=psQ))
                fillers.extend(quarter_drain_closures(1))
                s_loop(2, 2)
                fillers.extend(quarter_drain_closures(2))
                fillers.extend(quarter_drain_closures(3))
                s_loop(3, [3] * 6 + [2] * 6 + [1] * 4)
                emit_fill(len(fillers))

    nc.compile()
    return nc


def _host_inputs(x, pos_bias, w_qkv, w_out):
    """Build the per-core input maps (head-parallel sharding)."""
    bf = ml_dtypes.bfloat16
    x = np.asarray(x, dtype=np.float32)
    pos_bias = np.asarray(pos_bias, dtype=np.float32)
    w_qkv = np.asarray(w_qkv, dtype=np.float32)
    w_out = np.asarray(w_out, dtype=np.float32)
    hidden = HEADS * DIM_HEAD

    xt = np.ascontiguousarray(
        np.concatenate([x[0].T, x[1].T], axis=1)).astype(bf)  # [512, 4096]

    inv_freq = 1.0 / (ROPE_THETA ** (np.arange(0, DIM_HEAD, 2, dtype=np.float64) / DIM_HEAD))
    freqs = np.arange(N, dtype=np.float64)[:, None] * inv_freq[None, :]
    freqs = np.repeat(freqs, 2, axis=-1)  # [n, 64]
    cosT = np.cos(freqs).T.astype(np.float32)
    sinT = np.sin(freqs).T.astype(np.float32)
    cs2 = np.ascontiguousarray(np.concatenate([cosT, sinT], axis=0))  # [128, n]

    def rot_cols(w):
        wr = np.empty_like(w)
        wr[:, 0::2] = -w[:, 1::2]
        wr[:, 1::2] = w[:, 0::2]
        return wr

    scale = DIM_HEAD ** -0.5
    in_maps = []
    for h in range(HEADS):
        wq = w_qkv[:, h * 64:(h + 1) * 64] * scale
        wk = w_qkv[:, hidden + h * 64:hidden + (h + 1) * 64]
        wvh = w_qkv[:, 2 * hidden + h * 64:2 * hidden + (h + 1) * 64]
        wall = np.ascontiguousarray(
            np.concatenate(
                [wq, rot_cols(wq), wk, rot_cols(wk), wvh,
                 np.zeros((DIM, 64), dtype=np.float32)], axis=1)
        ).astype(bf)  # [512, 384]
        in_maps.append({
            "xt": xt,
            "wall": wall,
            "cs2": cs2,
            "ebt": np.ascontiguousarray(np.exp(pos_bias[h]).T).astype(bf),
        })
    return in_maps


def kernel(x, pos_bias, w_qkv, w_out, _want_trace=False):
    global _compiled
    from concourse.bass_utils import run_bass_kernel_spmd

    if _compiled is None:
        _compiled = _build()
    in_maps = _host_inputs(x, pos_bias, w_qkv, w_out)
    res = run_bass_kernel_spmd(
        _compiled, in_maps, core_ids=list(range(HEADS)), trace=_want_trace
    )
    w_out = np.asarray(w_out, dtype=np.float32)
    y = np.zeros((B, N, DIM), dtype=np.float32)
    for h, r in enumerate(res.results):
        rs = np.asarray(r["rsum"]).reshape(B, N)
        # oto: [4b+q, 64 d, 512 tok] -> O [B, N, 64]
        ot = np.asarray(r["oto"]).astype(np.float32)
        O = ot.reshape(B, 4, 64, 512).transpose(0, 1, 3, 2).reshape(B, N, 64)
        y += (O / rs[:, :, None]) @ w_out[h * 64:(h + 1) * 64, :]
    if _want_trace:
        kernel._last_results = res
    return y
